# revision 6
# baseline (speedup 1.0000x reference)
"""Trainium2 Bass kernel for nn_ConditionedMADE.

Reference computes a 16-step jax.lax.scan; step i runs a full 3-layer masked
MLP (MADE) pass and writes x[:, i] = input[:, i]*exp(a[:, i]) + m[:, i].
Returns (x, a_final.sum(-1)).

Key structure exploited here:
- The MADE masks are autoregressive in hidden-unit *degree* (deg = idx % 15).
  Sorting hidden units by degree (a host-side permutation of both hidden
  layers, which the network is invariant to) makes step i depend only on the
  prefix of units with deg <= i-1, and step i adds exactly one new degree
  "band" (~68 units) whose value equals its value in the final pass. So the
  16 full passes collapse into 15 incremental band updates plus a running
  32-row output accumulator — ~30x less matmul work.
- cond @ Wc.T is step-invariant: computed once per band.
- Bands are zero-padded to 128 rows so every SBUF/PSUM partition access is
  base-0 / 32-aligned (this walrus build rejects unaligned partition starts).

Data parallel over batch: 2048 rows -> 8 cores x 256. Activations are
feature-major [feature_partition, batch_free] so every matmul has N=256
(full-rate float32r). Row extraction (a_i, m_i) is done with selector
matmuls; x rows are re-packed with rank-1 matmuls into a PSUM accumulator.
"""

import sys

sys.path.insert(0, "/opt/trn_rl_repo")
sys.path.insert(0, "/root/.axon_site")

import numpy as np

N = 16  # autoregressive inputs
H = 1024  # hidden units
C = 128  # cond features
NB = N - 1  # 15 degree bands
PB = 128  # band padding
HP = NB * PB  # padded hidden size 1920
NCORES = 8
BATCH = 2048
BC = BATCH // NCORES  # 256 per core

_COMPILED = {}


def _host_prep(W1, b1, Wc, W2, b2, W3, b3):
    """Mask, degree-sort, band-pad and transpose all weights (numpy, f32)."""
    f32 = np.float32
    deg_in = np.arange(N) % N
    deg_hid = np.arange(H) % NB
    deg_out = np.arange(2 * N) % N - 1
    m1 = (deg_hid[:, None] >= deg_in[None, :]).astype(f32)
    m2 = (deg_hid[:, None] >= deg_hid[None, :]).astype(f32)
    m3 = (deg_out[:, None] >= deg_hid[None, :]).astype(f32)

    perm = np.argsort(deg_hid, kind="stable")
    counts = np.bincount(deg_hid[perm], minlength=NB)

    W1s = (W1 * m1)[perm, :].astype(f32)  # (H, 16)
    b1s = b1[perm].astype(f32)
    Wcs = Wc[perm, :].astype(f32)  # (H, C)
    W2s = (W2 * m2)[perm][:, perm].astype(f32)  # (H, H)
    b2s = b2[perm].astype(f32)
    W3s = (W3 * m3)[:, perm].astype(f32)  # (32, H)

    # scatter sorted rows into 128-padded band slots
    pad_idx = np.concatenate(
        [np.arange(counts[d]) + PB * d for d in range(NB)]
    )  # position of sorted row k in padded layout
    W1p = np.zeros((HP, N), f32)
    W1p[pad_idx] = W1s
    b1p = np.zeros(HP, f32)
    b1p[pad_idx] = b1s
    Wcp = np.zeros((HP, C), f32)
    Wcp[pad_idx] = Wcs
    b2p = np.zeros(HP, f32)
    b2p[pad_idx] = b2s
    W2p = np.zeros((HP, HP), f32)
    W2p[np.ix_(pad_idx, pad_idx)] = W2s
    W3p = np.zeros((2 * N, HP), f32)
    W3p[:, pad_idx] = W3s

    id32 = np.eye(32, dtype=f32)
    e16flat = np.eye(N, dtype=f32).reshape(1, N * N)  # row i at cols 16i..16i+16
    amask = np.zeros((32, 1), f32)
    amask[N:, 0] = 1.0

    return {
        "w1pT": W1p.T.copy(),  # (16, HP)
        "wcpT": Wcp.T.copy(),  # (C, HP)
        "w2pT": W2p.T.copy(),  # (HP, HP): [k_feature, band_col]
        "w3pT": W3p.T.copy(),  # (HP, 32)
        "b1p": b1p.reshape(1, HP),
        "b2cols": b2p.reshape(NB, PB).T.copy(),  # (128, NB)
        "b3row": b3.astype(f32).reshape(1, 2 * N),
        "id32": id32,
        "e16flat": e16flat,
        "amask": amask,
        "onesrow": np.ones((1, BC), f32),
    }


def _build(use_f32r=True):
    import concourse.bass as bass
    import concourse.tile as tile
    from concourse import mybir

    f32 = mybir.dt.float32
    f32r = mybir.dt.float32r
    AF = mybir.ActivationFunctionType

    md = f32r if use_f32r else f32  # dtype for matmul operands

    def mc(ap):
        return ap

    nc = bass.Bass()
    # per-core inputs (feature-major shards)
    inT = nc.dram_tensor("inT", (N, BC), f32, kind="ExternalInput")
    condT = nc.dram_tensor("condT", (C, BC), md, kind="ExternalInput")
    # replicated preprocessed weights
    w1pT = nc.dram_tensor("w1pT", (N, HP), md, kind="ExternalInput")
    wcpT = nc.dram_tensor("wcpT", (C, HP), md, kind="ExternalInput")
    w2pT = nc.dram_tensor("w2pT", (HP, HP), md, kind="ExternalInput")
    w3pT = nc.dram_tensor("w3pT", (HP, 2 * N), md, kind="ExternalInput")
    b1p = nc.dram_tensor("b1p", (1, HP), md, kind="ExternalInput")
    b2cols = nc.dram_tensor("b2cols", (PB, NB), f32, kind="ExternalInput")
    b3row = nc.dram_tensor("b3row", (1, 2 * N), md, kind="ExternalInput")
    id32 = nc.dram_tensor("id32", (32, 32), md, kind="ExternalInput")
    e16flat = nc.dram_tensor("e16flat", (1, N * N), md, kind="ExternalInput")
    amask = nc.dram_tensor("amask", (32, 1), md, kind="ExternalInput")
    onesrow = nc.dram_tensor("onesrow", (1, BC), md, kind="ExternalInput")
    # outputs
    xT_out = nc.dram_tensor("xT_out", (N, BC), md, kind="ExternalOutput")
    asum_out = nc.dram_tensor("asum_out", (1, BC), f32, kind="ExternalOutput")

    from contextlib import ExitStack

    with tile.TileContext(nc) as tc, ExitStack() as ctx:
        consts = ctx.enter_context(tc.tile_pool(name="consts", bufs=1))
        w2pool = ctx.enter_context(tc.tile_pool(name="w2", bufs=1))
        h1pool = ctx.enter_context(tc.tile_pool(name="h1", bufs=1))
        h2pool = ctx.enter_context(tc.tile_pool(name="h2sb", bufs=2))
        oapool = ctx.enter_context(tc.tile_pool(name="oasb", bufs=2))
        smpool = ctx.enter_context(tc.tile_pool(name="small", bufs=4))
        xtpool = ctx.enter_context(tc.tile_pool(name="xtsb", bufs=2))
        ps_oa = ctx.enter_context(tc.tile_pool(name="ps_oa", bufs=1, space="PSUM"))
        ps_xt = ctx.enter_context(tc.tile_pool(name="ps_xt", bufs=1, space="PSUM"))
        ps_h1 = ctx.enter_context(tc.tile_pool(name="ps_h1", bufs=2, space="PSUM"))
        ps_h2 = ctx.enter_context(tc.tile_pool(name="ps_h2", bufs=2, space="PSUM"))
        ps_am = ctx.enter_context(tc.tile_pool(name="ps_am", bufs=2, space="PSUM"))

        # ---- constants / inputs to SBUF ----
        condT_sb = consts.tile([C, BC], md, tag="condT")
        nc.sync.dma_start(out=condT_sb, in_=condT[:, :])
        in_rows = []
        for i in range(N):
            t = consts.tile([1, BC], f32, tag=f"in{i}")
            nc.sync.dma_start(out=t, in_=inT[i : i + 1, :])
            in_rows.append(t)
        w1_sb = consts.tile([N, HP], md, tag="w1")
        nc.sync.dma_start(out=w1_sb, in_=w1pT[:, :])
        wc_sb = consts.tile([C, HP], md, tag="wc")
        nc.sync.dma_start(out=wc_sb, in_=wcpT[:, :])
        b1_sb = consts.tile([1, HP], md, tag="b1")
        nc.sync.dma_start(out=b1_sb, in_=b1p[:, :])
        b2_sb = consts.tile([PB, NB], f32, tag="b2")
        nc.sync.dma_start(out=b2_sb, in_=b2cols[:, :])
        b3_sb = consts.tile([1, 2 * N], md, tag="b3")
        nc.sync.dma_start(out=b3_sb, in_=b3row[:, :])
        id32_sb = consts.tile([32, 32], md, tag="id32")
        nc.sync.dma_start(out=id32_sb, in_=id32[:, :])
        e16_sb = consts.tile([1, N * N], md, tag="e16")
        nc.sync.dma_start(out=e16_sb, in_=e16flat[:, :])
        am_sb = consts.tile([32, 1], md, tag="amask")
        nc.sync.dma_start(out=am_sb, in_=amask[:, :])
        ones_sb = consts.tile([1, BC], md, tag="ones")
        nc.sync.dma_start(out=ones_sb, in_=onesrow[:, :])

        # w3 k-tiles
        w3_sb = []
        for d in range(NB):
            t = consts.tile([PB, 2 * N], md, tag=f"w3_{d}")
            nc.sync.dma_start(out=t, in_=w3pT[PB * d : PB * (d + 1), :])
            w3_sb.append(t)

        # w2 k-tiles, streamed in column-block (deadline) order
        w2_sb = [
            w2pool.tile([PB, HP], md, tag=f"w2_{d}", name=f"w2_{d}")
            for d in range(NB)
        ]
        for c in range(NB):
            for d in range(c + 1):
                nc.sync.dma_start(
                    out=w2_sb[d][:, PB * c : PB * (c + 1)],
                    in_=w2pT[PB * d : PB * (d + 1), PB * c : PB * (c + 1)],
                )

        # ---- out_acc init: out_acc[j, b] = b3[j] ----
        out_acc = ps_oa.tile([32, BC], f32, tag="oa")
        nc.tensor.matmul(
            out_acc, mc(b3_sb), mc(ones_sb), start=True, stop=False,
            skip_group_check=True,
        )

        xT_ps = ps_xt.tile([N, BC], f32, tag="xt")
        h1_sb = [None] * NB
        xT_sb_prev = None

        def extract_and_update(i, last):
            """Read out_acc rows (i, 16+i), update x row i, accumulate xT."""
            nonlocal xT_sb_prev
            oa_sb = oapool.tile([32, BC], md, tag="oa_sb")
            nc.vector.tensor_copy(out=oa_sb, in_=out_acc)
            a_ps = ps_am.tile([1, BC], f32, tag="am")
            nc.tensor.matmul(
                a_ps, mc(id32_sb[:, N + i : N + i + 1]), mc(oa_sb),
                start=True, stop=True, skip_group_check=True,
            )
            m_ps = ps_am.tile([1, BC], f32, tag="am")
            nc.tensor.matmul(
                m_ps, mc(id32_sb[:, i : i + 1]), mc(oa_sb),
                start=True, stop=True, skip_group_check=True,
            )
            e_sb = smpool.tile([1, BC], f32, tag="e")
            nc.scalar.activation(out=e_sb, in_=a_ps, func=AF.Exp)
            xrow = smpool.tile([1, BC], md, tag="xrow")
            nc.vector.tensor_mul(out=xrow, in0=e_sb, in1=in_rows[i])
            nc.vector.tensor_add(out=xrow, in0=xrow, in1=m_ps)
            nc.tensor.matmul(
                xT_ps, mc(e16_sb[:, N * i : N * (i + 1)]), mc(xrow),
                start=(i == 0), stop=last, skip_group_check=True,
            )
            xT_sb = xtpool.tile([N, BC], md, tag="xT")
            nc.vector.tensor_copy(out=xT_sb, in_=xT_ps)
            xT_sb_prev = xT_sb
            if last:
                asum_ps = ps_am.tile([1, BC], f32, tag="am")
                nc.tensor.matmul(
                    asum_ps, mc(am_sb), mc(oa_sb),
                    start=True, stop=True, skip_group_check=True,
                )
                asum_sb = smpool.tile([1, BC], f32, tag="asum")
                nc.vector.tensor_copy(out=asum_sb, in_=asum_ps)
                nc.sync.dma_start(out=asum_out[:, :], in_=asum_sb)
                nc.sync.dma_start(out=xT_out[:, :], in_=xT_sb)

        # step 0: x[:,0] from bias-only out_acc
        extract_and_update(0, False)

        for i in range(1, N):
            d = i - 1  # the new band
            cs = slice(PB * d, PB * (d + 1))
            # h1 band d = relu(Wc_band@condT + b1_band + W1_band@x)
            h1b = ps_h1.tile([PB, BC], f32, tag="h1b")
            nc.tensor.matmul(
                h1b, mc(wc_sb[:, cs]), mc(condT_sb),
                start=True, stop=False, skip_group_check=True,
            )
            nc.tensor.matmul(
                h1b, mc(b1_sb[:, cs]), mc(ones_sb),
                start=False, stop=False, skip_group_check=True,
            )
            nc.tensor.matmul(
                h1b, mc(w1_sb[:, cs]), mc(xT_sb_prev),
                start=False, stop=True, skip_group_check=True,
            )
            h1d = h1pool.tile([PB, BC], md, tag=f"h1_{d}")
            nc.scalar.activation(out=h1d, in_=h1b, func=AF.Relu)
            h1_sb[d] = h1d
            # h2 band d = relu(W2[band, :prefix]@h1[:prefix] + b2_band)
            h2b = ps_h2.tile([PB, BC], f32, tag="h2b")
            for e in range(i):
                nc.tensor.matmul(
                    h2b, mc(w2_sb[e][:, cs]), mc(h1_sb[e]),
                    start=(e == 0), stop=(e == d), skip_group_check=True,
                )
            h2d = h2pool.tile([PB, BC], md, tag="h2sb")
            nc.scalar.activation(
                out=h2d, in_=h2b, func=AF.Relu, bias=b2_sb[:, d : d + 1]
            )
            # out_acc += W3[:, band]@h2_band
            nc.tensor.matmul(
                out_acc, mc(w3_sb[d]), mc(h2d),
                start=False, stop=(i == N - 1), skip_group_check=True,
            )
            extract_and_update(i, i == N - 1)

    from waitfix import split_multi_waits

    split_multi_waits(nc)
    return nc


def _get_compiled(use_f32r=True):
    key = use_f32r
    if key not in _COMPILED:
        import drainfix

        drainfix.apply()
        _COMPILED[key] = _build(use_f32r)
    return _COMPILED[key]


def kernel(input, conditioned_on, W1, b1, Wc, W2, b2, W3, b3, _trace=False,
           _use_f32r=True):
    import drainfix

    drainfix.apply()
    if _trace:
        import ntff_shim

        ntff_shim.apply()
    from concourse.bass_utils import run_bass_kernel_spmd

    prep = _host_prep(
        np.asarray(W1, np.float32),
        np.asarray(b1, np.float32),
        np.asarray(Wc, np.float32),
        np.asarray(W2, np.float32),
        np.asarray(b2, np.float32),
        np.asarray(W3, np.float32),
        np.asarray(b3, np.float32),
    )
    inT_full = np.ascontiguousarray(np.asarray(input, np.float32).T)  # (16, 2048)
    condT_full = np.ascontiguousarray(
        np.asarray(conditioned_on, np.float32).T
    )  # (128, 2048)

    nc = _get_compiled(_use_f32r)
    in_maps = []
    for c in range(NCORES):
        sl = slice(BC * c, BC * (c + 1))
        m = dict(prep)
        m["inT"] = np.ascontiguousarray(inT_full[:, sl])
        m["condT"] = np.ascontiguousarray(condT_full[:, sl])
        in_maps.append(m)

    res = run_bass_kernel_spmd(
        nc, in_maps, core_ids=list(range(NCORES)), trace=_trace
    )
    x = np.concatenate(
        [r["xT_out"].T for r in res.results], axis=0
    ).astype(np.float32)
    asum = np.concatenate(
        [r["asum_out"][0] for r in res.results], axis=0
    ).astype(np.float32)
    if _trace:
        kernel._last_result = res
    return x, asum


# revision 7
# speedup vs baseline: 1.0083x; 1.0083x over previous
"""Trainium2 Bass kernel for nn_ConditionedMADE.

Reference computes a 16-step jax.lax.scan; step i runs a full 3-layer masked
MLP (MADE) pass and writes x[:, i] = input[:, i]*exp(a[:, i]) + m[:, i].
Returns (x, a_final.sum(-1)).

Key structure exploited here:
- The MADE masks are autoregressive in hidden-unit *degree* (deg = idx % 15).
  Sorting hidden units by degree (a host-side permutation of both hidden
  layers, which the network is invariant to) makes step i depend only on the
  prefix of units with deg <= i-1, and step i adds exactly one new degree
  "band" (~68 units) whose value equals its value in the final pass. So the
  16 full passes collapse into 15 incremental band updates plus a running
  32-row output accumulator — ~30x less matmul work.
- cond @ Wc.T is step-invariant: computed once per band.
- Bands are zero-padded to 128 rows so every SBUF/PSUM partition access is
  base-0 / 32-aligned (this walrus build rejects unaligned partition starts).

Data parallel over batch: 2048 rows -> 8 cores x 256. Activations are
feature-major [feature_partition, batch_free] so every matmul has N=256
(full-rate float32r). Row extraction (a_i, m_i) is done with selector
matmuls; x rows are re-packed with rank-1 matmuls into a PSUM accumulator.
"""

import sys

sys.path.insert(0, "/opt/trn_rl_repo")
sys.path.insert(0, "/root/.axon_site")

import numpy as np

N = 16  # autoregressive inputs
H = 1024  # hidden units
C = 128  # cond features
NB = N - 1  # 15 degree bands
PB = 128  # band padding
HP = NB * PB  # padded hidden size 1920
NCORES = 8
BATCH = 2048
BC = BATCH // NCORES  # 256 per core

_COMPILED = {}


def _host_prep(W1, b1, Wc, W2, b2, W3, b3):
    """Mask, degree-sort, band-pad and transpose all weights (numpy, f32)."""
    f32 = np.float32
    deg_in = np.arange(N) % N
    deg_hid = np.arange(H) % NB
    deg_out = np.arange(2 * N) % N - 1
    m1 = (deg_hid[:, None] >= deg_in[None, :]).astype(f32)
    m2 = (deg_hid[:, None] >= deg_hid[None, :]).astype(f32)
    m3 = (deg_out[:, None] >= deg_hid[None, :]).astype(f32)

    perm = np.argsort(deg_hid, kind="stable")
    counts = np.bincount(deg_hid[perm], minlength=NB)

    W1s = (W1 * m1)[perm, :].astype(f32)  # (H, 16)
    b1s = b1[perm].astype(f32)
    Wcs = Wc[perm, :].astype(f32)  # (H, C)
    W2s = (W2 * m2)[perm][:, perm].astype(f32)  # (H, H)
    b2s = b2[perm].astype(f32)
    W3s = (W3 * m3)[:, perm].astype(f32)  # (32, H)

    # scatter sorted rows into 128-padded band slots
    pad_idx = np.concatenate(
        [np.arange(counts[d]) + PB * d for d in range(NB)]
    )  # position of sorted row k in padded layout
    W1p = np.zeros((HP, N), f32)
    W1p[pad_idx] = W1s
    b1p = np.zeros(HP, f32)
    b1p[pad_idx] = b1s
    Wcp = np.zeros((HP, C), f32)
    Wcp[pad_idx] = Wcs
    b2p = np.zeros(HP, f32)
    b2p[pad_idx] = b2s
    W2p = np.zeros((HP, HP), f32)
    W2p[np.ix_(pad_idx, pad_idx)] = W2s
    W3p = np.zeros((2 * N, HP), f32)
    W3p[:, pad_idx] = W3s

    id32 = np.eye(32, dtype=f32)
    e16flat = np.eye(N, dtype=f32).reshape(1, N * N)  # row i at cols 16i..16i+16
    amask = np.zeros((32, 1), f32)
    amask[N:, 0] = 1.0

    return {
        "w1pT": W1p.T.copy(),  # (16, HP)
        "wcpT": Wcp.T.copy(),  # (C, HP)
        "w2pT": W2p.T.copy(),  # (HP, HP): [k_feature, band_col]
        "w3pT": W3p.T.copy(),  # (HP, 32)
        "b1p": b1p.reshape(1, HP),
        "b2cols": b2p.reshape(NB, PB).T.copy(),  # (128, NB)
        "b3row": b3.astype(f32).reshape(1, 2 * N),
        "id32": id32,
        "e16flat": e16flat,
        "amask": amask,
        "onesrow": np.ones((1, BC), f32),
    }


def _build(use_f32r=True):
    import concourse.bass as bass
    import concourse.tile as tile
    from concourse import mybir

    f32 = mybir.dt.float32
    f32r = mybir.dt.float32r
    AF = mybir.ActivationFunctionType

    md = f32r if use_f32r else f32  # dtype for matmul operands

    def mc(ap):
        return ap

    nc = bass.Bass()
    # per-core inputs (feature-major shards)
    inT = nc.dram_tensor("inT", (N, BC), f32, kind="ExternalInput")
    condT = nc.dram_tensor("condT", (C, BC), md, kind="ExternalInput")
    # replicated preprocessed weights
    w1pT = nc.dram_tensor("w1pT", (N, HP), md, kind="ExternalInput")
    wcpT = nc.dram_tensor("wcpT", (C, HP), md, kind="ExternalInput")
    w2pT = nc.dram_tensor("w2pT", (HP, HP), md, kind="ExternalInput")
    w3pT = nc.dram_tensor("w3pT", (HP, 2 * N), md, kind="ExternalInput")
    b1p = nc.dram_tensor("b1p", (1, HP), md, kind="ExternalInput")
    b2cols = nc.dram_tensor("b2cols", (PB, NB), f32, kind="ExternalInput")
    b3row = nc.dram_tensor("b3row", (1, 2 * N), md, kind="ExternalInput")
    id32 = nc.dram_tensor("id32", (32, 32), md, kind="ExternalInput")
    e16flat = nc.dram_tensor("e16flat", (1, N * N), md, kind="ExternalInput")
    amask = nc.dram_tensor("amask", (32, 1), md, kind="ExternalInput")
    onesrow = nc.dram_tensor("onesrow", (1, BC), md, kind="ExternalInput")
    # outputs
    xT_out = nc.dram_tensor("xT_out", (N, BC), md, kind="ExternalOutput")
    asum_out = nc.dram_tensor("asum_out", (1, BC), f32, kind="ExternalOutput")

    from contextlib import ExitStack

    with tile.TileContext(nc) as tc, ExitStack() as ctx:
        consts = ctx.enter_context(tc.tile_pool(name="consts", bufs=1))
        w2pool = ctx.enter_context(tc.tile_pool(name="w2", bufs=1))
        h1pool = ctx.enter_context(tc.tile_pool(name="h1", bufs=1))
        h2pool = ctx.enter_context(tc.tile_pool(name="h2sb", bufs=2))
        oapool = ctx.enter_context(tc.tile_pool(name="oasb", bufs=2))
        smpool = ctx.enter_context(tc.tile_pool(name="small", bufs=4))
        xtpool = ctx.enter_context(tc.tile_pool(name="xtsb", bufs=2))
        ps_oa = ctx.enter_context(tc.tile_pool(name="ps_oa", bufs=1, space="PSUM"))
        ps_xt = ctx.enter_context(tc.tile_pool(name="ps_xt", bufs=1, space="PSUM"))
        ps_h1 = ctx.enter_context(tc.tile_pool(name="ps_h1", bufs=2, space="PSUM"))
        ps_h2 = ctx.enter_context(tc.tile_pool(name="ps_h2", bufs=2, space="PSUM"))
        ps_am = ctx.enter_context(tc.tile_pool(name="ps_am", bufs=2, space="PSUM"))

        # ---- constants / inputs to SBUF ----
        condT_sb = consts.tile([C, BC], md, tag="condT")
        nc.sync.dma_start(out=condT_sb, in_=condT[:, :])
        in_rows = []
        for i in range(N):
            t = consts.tile([1, BC], f32, tag=f"in{i}")
            nc.sync.dma_start(out=t, in_=inT[i : i + 1, :])
            in_rows.append(t)
        w1_sb = consts.tile([N, HP], md, tag="w1")
        nc.sync.dma_start(out=w1_sb, in_=w1pT[:, :])
        wc_sb = consts.tile([C, HP], md, tag="wc")
        nc.gpsimd.dma_start(out=wc_sb, in_=wcpT[:, :])
        b1_sb = consts.tile([1, HP], md, tag="b1")
        nc.sync.dma_start(out=b1_sb, in_=b1p[:, :])
        b2_sb = consts.tile([PB, NB], f32, tag="b2")
        nc.sync.dma_start(out=b2_sb, in_=b2cols[:, :])
        b3_sb = consts.tile([1, 2 * N], md, tag="b3")
        nc.sync.dma_start(out=b3_sb, in_=b3row[:, :])
        id32_sb = consts.tile([32, 32], md, tag="id32")
        nc.sync.dma_start(out=id32_sb, in_=id32[:, :])
        e16_sb = consts.tile([1, N * N], md, tag="e16")
        nc.sync.dma_start(out=e16_sb, in_=e16flat[:, :])
        am_sb = consts.tile([32, 1], md, tag="amask")
        nc.sync.dma_start(out=am_sb, in_=amask[:, :])
        ones_sb = consts.tile([1, BC], md, tag="ones")
        nc.sync.dma_start(out=ones_sb, in_=onesrow[:, :])

        # w3 k-tiles
        w3_sb = []
        for d in range(NB):
            t = consts.tile([PB, 2 * N], md, tag=f"w3_{d}")
            nc.gpsimd.dma_start(out=t, in_=w3pT[PB * d : PB * (d + 1), :])
            w3_sb.append(t)

        # w2 k-tiles, streamed in column-block (deadline) order
        w2_sb = [
            w2pool.tile([PB, HP], md, tag=f"w2_{d}", name=f"w2_{d}")
            for d in range(NB)
        ]
        for c in range(NB):
            for d in range(c + 1):
                nc.gpsimd.dma_start(
                    out=w2_sb[d][:, PB * c : PB * (c + 1)],
                    in_=w2pT[PB * d : PB * (d + 1), PB * c : PB * (c + 1)],
                )

        # ---- out_acc init: out_acc[j, b] = b3[j] ----
        out_acc = ps_oa.tile([32, BC], f32, tag="oa")
        nc.tensor.matmul(
            out_acc, mc(b3_sb), mc(ones_sb), start=True, stop=False,
            skip_group_check=True,
        )

        xT_ps = ps_xt.tile([N, BC], f32, tag="xt")
        h1_sb = [None] * NB
        xT_sb_prev = None

        def extract_and_update(i, last):
            """Read out_acc rows (i, 16+i), update x row i, accumulate xT."""
            nonlocal xT_sb_prev
            oa_sb = oapool.tile([32, BC], md, tag="oa_sb")
            nc.vector.tensor_copy(out=oa_sb, in_=out_acc)
            a_ps = ps_am.tile([1, BC], f32, tag="am")
            nc.tensor.matmul(
                a_ps, mc(id32_sb[:, N + i : N + i + 1]), mc(oa_sb),
                start=True, stop=True, skip_group_check=True,
            )
            m_ps = ps_am.tile([1, BC], f32, tag="am")
            nc.tensor.matmul(
                m_ps, mc(id32_sb[:, i : i + 1]), mc(oa_sb),
                start=True, stop=True, skip_group_check=True,
            )
            e_sb = smpool.tile([1, BC], f32, tag="e")
            nc.scalar.activation(out=e_sb, in_=a_ps, func=AF.Exp)
            xrow = smpool.tile([1, BC], md, tag="xrow")
            nc.vector.tensor_mul(out=xrow, in0=e_sb, in1=in_rows[i])
            nc.vector.tensor_add(out=xrow, in0=xrow, in1=m_ps)
            nc.tensor.matmul(
                xT_ps, mc(e16_sb[:, N * i : N * (i + 1)]), mc(xrow),
                start=(i == 0), stop=last, skip_group_check=True,
            )
            xT_sb = xtpool.tile([N, BC], md, tag="xT")
            nc.vector.tensor_copy(out=xT_sb, in_=xT_ps)
            xT_sb_prev = xT_sb
            if last:
                asum_ps = ps_am.tile([1, BC], f32, tag="am")
                nc.tensor.matmul(
                    asum_ps, mc(am_sb), mc(oa_sb),
                    start=True, stop=True, skip_group_check=True,
                )
                asum_sb = smpool.tile([1, BC], f32, tag="asum")
                nc.vector.tensor_copy(out=asum_sb, in_=asum_ps)
                nc.sync.dma_start(out=asum_out[:, :], in_=asum_sb)
                nc.sync.dma_start(out=xT_out[:, :], in_=xT_sb)

        # step 0: x[:,0] from bias-only out_acc
        extract_and_update(0, False)

        for i in range(1, N):
            d = i - 1  # the new band
            cs = slice(PB * d, PB * (d + 1))
            # h1 band d = relu(Wc_band@condT + b1_band + W1_band@x)
            h1b = ps_h1.tile([PB, BC], f32, tag="h1b")
            nc.tensor.matmul(
                h1b, mc(wc_sb[:, cs]), mc(condT_sb),
                start=True, stop=False, skip_group_check=True,
            )
            nc.tensor.matmul(
                h1b, mc(b1_sb[:, cs]), mc(ones_sb),
                start=False, stop=False, skip_group_check=True,
            )
            nc.tensor.matmul(
                h1b, mc(w1_sb[:, cs]), mc(xT_sb_prev),
                start=False, stop=True, skip_group_check=True,
            )
            h1d = h1pool.tile([PB, BC], md, tag=f"h1_{d}")
            nc.scalar.activation(out=h1d, in_=h1b, func=AF.Relu)
            h1_sb[d] = h1d
            # h2 band d = relu(W2[band, :prefix]@h1[:prefix] + b2_band)
            h2b = ps_h2.tile([PB, BC], f32, tag="h2b")
            for e in range(i):
                nc.tensor.matmul(
                    h2b, mc(w2_sb[e][:, cs]), mc(h1_sb[e]),
                    start=(e == 0), stop=(e == d), skip_group_check=True,
                )
            h2d = h2pool.tile([PB, BC], md, tag="h2sb")
            nc.scalar.activation(
                out=h2d, in_=h2b, func=AF.Relu, bias=b2_sb[:, d : d + 1]
            )
            # out_acc += W3[:, band]@h2_band
            nc.tensor.matmul(
                out_acc, mc(w3_sb[d]), mc(h2d),
                start=False, stop=(i == N - 1), skip_group_check=True,
            )
            extract_and_update(i, i == N - 1)

    from waitfix import split_multi_waits

    split_multi_waits(nc)
    return nc


def _get_compiled(use_f32r=True):
    key = use_f32r
    if key not in _COMPILED:
        import drainfix

        drainfix.apply()
        _COMPILED[key] = _build(use_f32r)
    return _COMPILED[key]


def kernel(input, conditioned_on, W1, b1, Wc, W2, b2, W3, b3, _trace=False,
           _use_f32r=True):
    import drainfix

    drainfix.apply()
    if _trace:
        import ntff_shim

        ntff_shim.apply()
    from concourse.bass_utils import run_bass_kernel_spmd

    prep = _host_prep(
        np.asarray(W1, np.float32),
        np.asarray(b1, np.float32),
        np.asarray(Wc, np.float32),
        np.asarray(W2, np.float32),
        np.asarray(b2, np.float32),
        np.asarray(W3, np.float32),
        np.asarray(b3, np.float32),
    )
    inT_full = np.ascontiguousarray(np.asarray(input, np.float32).T)  # (16, 2048)
    condT_full = np.ascontiguousarray(
        np.asarray(conditioned_on, np.float32).T
    )  # (128, 2048)

    nc = _get_compiled(_use_f32r)
    in_maps = []
    for c in range(NCORES):
        sl = slice(BC * c, BC * (c + 1))
        m = dict(prep)
        m["inT"] = np.ascontiguousarray(inT_full[:, sl])
        m["condT"] = np.ascontiguousarray(condT_full[:, sl])
        in_maps.append(m)

    res = run_bass_kernel_spmd(
        nc, in_maps, core_ids=list(range(NCORES)), trace=_trace
    )
    x = np.concatenate(
        [r["xT_out"].T for r in res.results], axis=0
    ).astype(np.float32)
    asum = np.concatenate(
        [r["asum_out"][0] for r in res.results], axis=0
    ).astype(np.float32)
    if _trace:
        kernel._last_result = res
    return x, asum
